# revision 5
# baseline (speedup 1.0000x reference)
"""MPNN-LSPE layer on 8 trn2 NeuronCores.

Strategy (edge-parallel, per sharding hint): edges are split into 8 equal
contiguous chunks.  The first MLP layer is linear, so it is algebraically
restructured into per-node projections computed once per node instead of
once per edge:

    state @ W1 = A[send] + B[rec] + dist * w1e,   A = x@W1a + pe@W1b, ...

The host computes the node projections, gathers them per edge, applies the
first activation, and ships h1 = silu(p1), hp1 = tanh(pp1) feature-major in
fp8 (e4m3).  Each core then runs the second (nonlinear) half of both edge
MLPs:

    matmul(W2, bf16) -> ACT Silu(psum + b2)  = msg   (bf16 out)
    matmul(Wp2)      -> ACT Tanh(psum + bp2) = msg_pe

streaming 2048-edge outer groups (one 512 KB fp8 DMA in, one 1 MB bf16 DMA
out per group).  The segment-sum aggregation + residual is done on host
(fp32).
"""

import os
import numpy as np
import ml_dtypes

import concourse.bass as bass
import concourse.mybir as mybir
import concourse.tile as tile
import bass_rust
from concourse.vector_clock import ScopedClock
from concourse.bass_utils import run_bass_kernel_spmd

N = 50000
E = 400000
H = 128
NCORES = 8
EC = E // NCORES          # 50000 edges per core
GE = 2048                 # edges per outer group (4 matmul chunks of 512)
NG = (EC + GE - 1) // GE  # 25 groups
EP = NG * GE              # 51200 padded edges per core
GH = 1024                 # edges per psum tile / act

F32 = mybir.dt.float32
BF16 = mybir.dt.bfloat16
FP8 = mybir.dt.float8e4

NPF8 = ml_dtypes.float8_e4m3
NPBF = ml_dtypes.bfloat16


def _patch_tail_drain():
    """Walrus rejects >2 sync waits on one instruction; the Tile tail drain
    accumulates one wait per outstanding sem. Spread them over SP nops."""
    def _split_drain_and_barrier(self, tick_clock, wait_clock):
        nc = self.nc
        spills = [nc.sync.nop(nofuse=True) for _ in range(24)]
        drain_inst = nc.sync.drain()
        wait_clock.add_sem_waits(
            drain_inst.ins, ScopedClock({None: tick_clock.global_clock})
        )
        si = drain_inst.ins.sync_info
        waits = list(si.on_wait) if si is not None else []
        if len(waits) > 1:
            si.on_wait = waits[:1]
            rest = waits[1:]
            assert len(rest) <= len(spills)
            for w, sp in zip(rest, spills):
                sp.ins.sync_info = bass_rust.SyncInfo(on_wait=[w], on_update=[])
        nc.all_engine_barrier()
        popped = nc._tile_sem_poison_stack.pop()
        assert popped is self._sem_poison
        nc.clear_and_free_semaphores(list(self.sems.allocated().values()))
        nc.all_engine_barrier()

    tile.TileContext._drain_and_barrier = _split_drain_and_barrier


def _split_excess_waits(nc, max_waits=1):
    """Walrus codegen caps embedded sync-wait commands per instruction; hoist
    excess waits onto same-engine no-ops inserted just before the inst."""
    for fn in nc.m.functions:
        for blk in fn.blocks:
            new_insts = []
            for inst in blk.instructions:
                si = inst.sync_info
                waits = list(si.on_wait) if si is not None else []
                if len(waits) > max_waits:
                    keep = waits[:max_waits]
                    rest = waits[max_waits:]
                    for k in range(0, len(rest), max_waits):
                        nop = mybir.InstNoOp(
                            name=nc.get_next_instruction_name(),
                            engine=inst.engine,
                            ins=[], outs=[],
                            sync_info=bass_rust.SyncInfo(
                                on_wait=rest[k:k + max_waits], on_update=[]
                            ),
                        )
                        new_insts.append(nop)
                    si.on_wait = keep
                new_insts.append(inst)
            blk.instructions = new_insts


def _build_nc():
    nc = bass.Bass()
    # hcat row-block g: [128 features, 2*GE] = h1 (cols 0:GE) | hp1 (GE:2GE)
    hcat = nc.dram_tensor("hcat", [NG * H, 2 * GE], FP8, kind="ExternalInput")
    wcat = nc.dram_tensor("wcat", [2 * H, H], BF16, kind="ExternalInput")
    biasT = nc.dram_tensor("biasT", [H, 2], F32, kind="ExternalInput")
    ocat = nc.dram_tensor("ocat", [NG * H, 2 * GE], BF16, kind="ExternalOutput")

    AF = mybir.ActivationFunctionType

    with tile.TileContext(nc) as tc:
        with tc.tile_pool(name="consts", bufs=1) as cpool, \
             tc.tile_pool(name="io", bufs=4) as iopool, \
             tc.tile_pool(name="out", bufs=4) as outpool, \
             tc.tile_pool(name="psm", bufs=2, space="PSUM") as psm, \
             tc.tile_pool(name="psp", bufs=2, space="PSUM") as psp:

            w2 = cpool.tile([H, H], BF16, tag="w2")
            nc.sync.dma_start(out=w2[:], in_=wcat[0:H, :])
            wp2 = cpool.tile([H, H], BF16, tag="wp2")
            nc.sync.dma_start(out=wp2[:], in_=wcat[H:2 * H, :])
            bias = cpool.tile([H, 2], F32, tag="bias")
            nc.sync.dma_start(out=bias[:], in_=biasT[:, :])

            for g in range(NG):
                hin = iopool.tile([H, 2 * GE], FP8, tag="hin")
                nc.sync.dma_start(
                    out=hin[:], in_=hcat[g * H:(g + 1) * H, :]
                )
                oout = outpool.tile([H, 2 * GE], BF16, tag="oout")

                for half in range(2):
                    mo = half * GH           # edge offset within the group
                    pm = psm.tile([H, GH], F32, tag="pm")
                    nc.tensor.matmul(
                        out=pm[:, 0:512], lhsT=w2[:],
                        rhs=hin[:, mo:mo + 512], start=True, stop=True)
                    nc.tensor.matmul(
                        out=pm[:, 512:GH], lhsT=w2[:],
                        rhs=hin[:, mo + 512:mo + GH], start=True, stop=True)

                    pp = psp.tile([H, GH], F32, tag="pp")
                    nc.tensor.matmul(
                        out=pp[:, 0:512], lhsT=wp2[:],
                        rhs=hin[:, GE + mo:GE + mo + 512],
                        start=True, stop=True)
                    nc.tensor.matmul(
                        out=pp[:, 512:GH], lhsT=wp2[:],
                        rhs=hin[:, GE + mo + 512:GE + mo + GH],
                        start=True, stop=True)

                    # silu path: DVE moves psum->sbuf with fused +b2; the
                    # silu itself is applied on host (out is pre-act).
                    nc.vector.tensor_scalar_add(
                        oout[:, mo:mo + GH], pm[:], bias[:, 0:1])
                    nc.scalar.activation(oout[:, GE + mo:GE + mo + GH], pp[:],
                                         AF.Tanh, bias=bias[:, 1:2])

                # output DMA on the ACT HWDGE ring so it overlaps with the
                # input stream on the SP ring
                nc.scalar.dma_start(
                    out=ocat[g * H:(g + 1) * H, :], in_=oout[:]
                )

    _split_excess_waits(nc)
    return nc


_CACHED = {}


def _silu(v):
    return v / (1.0 + np.exp(-v))


def kernel(x, pos, pe, edge_index, W1, b1, W2, b2, Wp1, bp1, Wp2, bp2):
    _patch_tail_drain()

    x = np.asarray(x, np.float32)
    pos = np.asarray(pos, np.float32)
    pe_a = np.asarray(pe, np.float32)
    ei = np.asarray(edge_index)
    send = ei[0].astype(np.int64)
    rec = ei[1].astype(np.int64)
    W1 = np.asarray(W1, np.float32); b1 = np.asarray(b1, np.float32)
    W2 = np.asarray(W2, np.float32); b2 = np.asarray(b2, np.float32)
    Wp1 = np.asarray(Wp1, np.float32); bp1 = np.asarray(bp1, np.float32)
    Wp2 = np.asarray(Wp2, np.float32); bp2 = np.asarray(bp2, np.float32)

    dist = np.sqrt(((pos[send] - pos[rec]) ** 2).sum(axis=1)).astype(np.float32)

    # first (linear) MLP layers as per-node projections
    A = x @ W1[0:H] + pe_a @ W1[H:2 * H]
    B = x @ W1[2 * H:3 * H] + pe_a @ W1[3 * H:4 * H]
    Ap = pe_a @ Wp1[0:H]
    Bp = pe_a @ Wp1[H:2 * H]

    p1 = A[send] + B[rec]
    p1 += dist[:, None] * W1[4 * H][None, :]
    p1 += b1
    h1 = _silu(p1).astype(NPF8)
    del p1
    pp1 = Ap[send] + Bp[rec]
    pp1 += dist[:, None] * Wp1[2 * H][None, :]
    pp1 += bp1
    hp1 = np.tanh(pp1).astype(NPF8)
    del pp1

    wcat = np.concatenate([W2, Wp2], axis=0).astype(NPBF)
    biasT = np.stack([b2, bp2], axis=1).astype(np.float32)  # [H,2]

    in_maps = []
    for c in range(NCORES):
        sl = slice(c * EC, (c + 1) * EC)
        hT = np.zeros((H, EP), NPF8)
        hT[:, :EC] = h1[sl].T
        hpT = np.zeros((H, EP), NPF8)
        hpT[:, :EC] = hp1[sl].T
        hcat = np.empty((NG, H, 2 * GE), NPF8)
        hcat[:, :, 0:GE] = hT.reshape(H, NG, GE).transpose(1, 0, 2)
        hcat[:, :, GE:2 * GE] = hpT.reshape(H, NG, GE).transpose(1, 0, 2)
        in_maps.append({"hcat": hcat.reshape(NG * H, 2 * GE),
                        "wcat": wcat, "biasT": biasT})

    if "nc" not in _CACHED:
        _CACHED["nc"] = _build_nc()
    nc = _CACHED["nc"]

    trace = bool(_CACHED.get("trace") or os.environ.get("KERNEL_TRACE"))
    res = run_bass_kernel_spmd(
        nc, in_maps, list(range(NCORES)), trace=trace,
        trace_cores=[0] if trace else None,
    )
    _CACHED["last_res"] = res

    msg = np.empty((E, H), np.float32)
    msgp = np.empty((E, H), np.float32)
    for c in range(NCORES):
        sl = slice(c * EC, (c + 1) * EC)
        oc = res.results[c]["ocat"].reshape(NG, H, 2 * GE)
        mT = oc[:, :, 0:GE].transpose(1, 0, 2).reshape(H, EP)
        mpT = oc[:, :, GE:2 * GE].transpose(1, 0, 2).reshape(H, EP)
        msg[sl] = _silu(mT[:, :EC].T.astype(np.float32))  # device sent pre-act
        msgp[sl] = mpT[:, :EC].T.astype(np.float32)

    # segment sum over rec (host, fp32)
    order = np.argsort(rec, kind="stable")
    rs = rec[order]
    starts = np.flatnonzero(np.r_[True, rs[1:] != rs[:-1]])
    uniq = rs[starts]
    aggr = np.zeros((N, H), np.float32)
    aggr[uniq] = np.add.reduceat(msg[order], starts, axis=0)
    aggr_pe = np.zeros((N, H), np.float32)
    aggr_pe[uniq] = np.add.reduceat(msgp[order], starts, axis=0)

    return x + aggr, pe_a + aggr_pe


# revision 6
# speedup vs baseline: 4.4168x; 4.4168x over previous
"""MPNN-LSPE layer on 8 trn2 NeuronCores.

Strategy (edge-parallel, per sharding hint): edges are sharded across the 8
cores.  The first MLP layer is linear, so it is algebraically restructured
into per-node projections computed once per node instead of once per edge:

    state @ W1 = A[send] + B[rec] + dist * w1e,   A = x@W1a + pe@W1b, ...

The host computes the node projections, gathers them per edge, applies the
first activation, and ships h1 = silu(p1), hp1 = tanh(pp1) feature-major in
fp8 (e4m3).  Edges are sorted by receiver and packed into even-sized
per-receiver slot runs so the device can pre-reduce pairs of messages that
share a receiver, cutting the output stream.  Each core then runs the
second half of both edge MLPs:

    silu path: matmul(W2, bf16) -> DVE +b2 -> pre-act out (host applies Silu)
    pe path:   matmul(Wp2)      -> ACT Tanh(psum + bp2) -> GPSIMD pair-add

Output per 2048-slot group: 2048 silu pre-act columns + 1024 tanh pair sums
(bf16), written on the ACT HWDGE ring while inputs stream on the SP ring.
The final segment-sum + residual runs on host (fp32).
"""

import os
import numpy as np
import ml_dtypes

import concourse.bass as bass
import concourse.mybir as mybir
import concourse.tile as tile
import bass_rust
from concourse.vector_clock import ScopedClock
from concourse.bass_utils import run_bass_kernel_spmd

N = 50000
E = 400000
H = 128
NCORES = 8
GE = 2048                 # slots per outer group
GB = 1024                 # slots per act block (= psum tile)
EPS = 26 * GE             # 53248 slots per core (fits E + even-padding)
NG = EPS // GE            # 26 groups
ST = NCORES * EPS         # total slots

F32 = mybir.dt.float32
BF16 = mybir.dt.bfloat16
FP8 = mybir.dt.float8e4

NPF8 = ml_dtypes.float8_e4m3
NPBF = ml_dtypes.bfloat16


def _patch_tail_drain():
    """Walrus rejects >2 sync waits on one instruction; the Tile tail drain
    accumulates one wait per outstanding sem. Spread them over SP nops."""
    def _split_drain_and_barrier(self, tick_clock, wait_clock):
        nc = self.nc
        spills = [nc.sync.nop(nofuse=True) for _ in range(24)]
        drain_inst = nc.sync.drain()
        wait_clock.add_sem_waits(
            drain_inst.ins, ScopedClock({None: tick_clock.global_clock})
        )
        si = drain_inst.ins.sync_info
        waits = list(si.on_wait) if si is not None else []
        if len(waits) > 1:
            si.on_wait = waits[:1]
            rest = waits[1:]
            assert len(rest) <= len(spills)
            for w, sp in zip(rest, spills):
                sp.ins.sync_info = bass_rust.SyncInfo(on_wait=[w], on_update=[])
        nc.all_engine_barrier()
        popped = nc._tile_sem_poison_stack.pop()
        assert popped is self._sem_poison
        nc.clear_and_free_semaphores(list(self.sems.allocated().values()))
        nc.all_engine_barrier()

    tile.TileContext._drain_and_barrier = _split_drain_and_barrier


def _split_excess_waits(nc, max_waits=1):
    """Walrus codegen caps embedded sync-wait commands per instruction; hoist
    excess waits onto same-engine no-ops inserted just before the inst."""
    for fn in nc.m.functions:
        for blk in fn.blocks:
            new_insts = []
            for inst in blk.instructions:
                si = inst.sync_info
                waits = list(si.on_wait) if si is not None else []
                if len(waits) > max_waits:
                    keep = waits[:max_waits]
                    rest = waits[max_waits:]
                    for k in range(0, len(rest), max_waits):
                        nop = mybir.InstNoOp(
                            name=nc.get_next_instruction_name(),
                            engine=inst.engine,
                            ins=[], outs=[],
                            sync_info=bass_rust.SyncInfo(
                                on_wait=rest[k:k + max_waits], on_update=[]
                            ),
                        )
                        new_insts.append(nop)
                    si.on_wait = keep
                new_insts.append(inst)
            blk.instructions = new_insts


def _build_nc():
    nc = bass.Bass()
    # hcat row-block g: [128 features, 2*GE] = h slots (0:GE) | hp slots
    hcat = nc.dram_tensor("hcat", [NG * H, 2 * GE], FP8, kind="ExternalInput")
    wcat = nc.dram_tensor("wcat", [2 * H, H], BF16, kind="ExternalInput")
    biasT = nc.dram_tensor("biasT", [H, 2], F32, kind="ExternalInput")
    # ocat row-block g: silu pre-acts (0:GE) | tanh pair sums (GE:GE+GE//2)
    OW = GE + GE // 2
    ocat = nc.dram_tensor("ocat", [NG * H, OW], BF16, kind="ExternalOutput")

    AF = mybir.ActivationFunctionType
    ADD = mybir.AluOpType.add

    with tile.TileContext(nc) as tc:
        with tc.tile_pool(name="consts", bufs=1) as cpool, \
             tc.tile_pool(name="io", bufs=6) as iopool, \
             tc.tile_pool(name="out", bufs=6) as outpool, \
             tc.tile_pool(name="mid", bufs=3) as midpool, \
             tc.tile_pool(name="psm", bufs=2, space="PSUM") as psm, \
             tc.tile_pool(name="psp", bufs=2, space="PSUM") as psp:

            w2 = cpool.tile([H, H], BF16, tag="w2")
            nc.sync.dma_start(out=w2[:], in_=wcat[0:H, :])
            wp2 = cpool.tile([H, H], BF16, tag="wp2")
            nc.sync.dma_start(out=wp2[:], in_=wcat[H:2 * H, :])
            bias = cpool.tile([H, 2], F32, tag="bias")
            nc.sync.dma_start(out=bias[:], in_=biasT[:, :])

            for g in range(NG):
                hin = iopool.tile([H, 2 * GE], FP8, tag="hin")
                nc.sync.dma_start(
                    out=hin[:], in_=hcat[g * H:(g + 1) * H, :]
                )
                oout = outpool.tile([H, OW], BF16, tag="oout")
                to = midpool.tile([H, GE], BF16, tag="to")

                for half in range(2):
                    mo = half * GB
                    # silu path: pre-act out (host applies Silu); +b2 on DVE
                    pm = psm.tile([H, GB], F32, tag="pm")
                    nc.tensor.matmul(
                        out=pm[:, 0:512], lhsT=w2[:],
                        rhs=hin[:, mo:mo + 512], start=True, stop=True)
                    nc.tensor.matmul(
                        out=pm[:, 512:GB], lhsT=w2[:],
                        rhs=hin[:, mo + 512:mo + GB], start=True, stop=True)
                    nc.vector.tensor_scalar_add(
                        oout[:, mo:mo + GB], pm[:], bias[:, 0:1])

                    # pe path: Tanh on ACT, then pair-add on GPSIMD
                    pp = psp.tile([H, GB], F32, tag="pp")
                    nc.tensor.matmul(
                        out=pp[:, 0:512], lhsT=wp2[:],
                        rhs=hin[:, GE + mo:GE + mo + 512],
                        start=True, stop=True)
                    nc.tensor.matmul(
                        out=pp[:, 512:GB], lhsT=wp2[:],
                        rhs=hin[:, GE + mo + 512:GE + mo + GB],
                        start=True, stop=True)
                    nc.scalar.activation(to[:, mo:mo + GB], pp[:],
                                         AF.Tanh, bias=bias[:, 1:2])
                    # pair q of this block = slots (q, q+512)
                    nc.gpsimd.tensor_tensor(
                        out=oout[:, GE + half * 512:GE + half * 512 + 512],
                        in0=to[:, mo:mo + 512],
                        in1=to[:, mo + 512:mo + GB],
                        op=ADD)

                # output DMA on the ACT HWDGE ring so it overlaps with the
                # input stream on the SP ring
                nc.scalar.dma_start(
                    out=ocat[g * H:(g + 1) * H, :], in_=oout[:]
                )

    _split_excess_waits(nc)
    return nc


_CACHED = {}


def _silu(v):
    return v / (1.0 + np.exp(-v))


def kernel(x, pos, pe, edge_index, W1, b1, W2, b2, Wp1, bp1, Wp2, bp2):
    _patch_tail_drain()

    x = np.asarray(x, np.float32)
    pos = np.asarray(pos, np.float32)
    pe_a = np.asarray(pe, np.float32)
    ei = np.asarray(edge_index)
    send = ei[0].astype(np.int64)
    rec = ei[1].astype(np.int64)
    W1 = np.asarray(W1, np.float32); b1 = np.asarray(b1, np.float32)
    W2 = np.asarray(W2, np.float32); b2 = np.asarray(b2, np.float32)
    Wp1 = np.asarray(Wp1, np.float32); bp1 = np.asarray(bp1, np.float32)
    Wp2 = np.asarray(Wp2, np.float32); bp2 = np.asarray(bp2, np.float32)

    dist = np.sqrt(((pos[send] - pos[rec]) ** 2).sum(axis=1)).astype(np.float32)

    # first (linear) MLP layers as per-node projections
    A = x @ W1[0:H] + pe_a @ W1[H:2 * H]
    B = x @ W1[2 * H:3 * H] + pe_a @ W1[3 * H:4 * H]
    Ap = pe_a @ Wp1[0:H]
    Bp = pe_a @ Wp1[H:2 * H]

    p1 = A[send] + B[rec]
    p1 += dist[:, None] * W1[4 * H][None, :]
    p1 += b1
    h1 = _silu(p1).astype(NPF8)
    del p1
    pp1 = Ap[send] + Bp[rec]
    pp1 += dist[:, None] * Wp1[2 * H][None, :]
    pp1 += bp1
    hp1 = np.tanh(pp1).astype(NPF8)
    del pp1

    # ---- receiver-sorted slot/pair layout ----
    # Each receiver's edges occupy ceil(c/2) consecutive PAIRS; pair q maps
    # to slots (o, o+512) of 1024-slot act block b, with q = b*512 + o, so
    # the device pair-add is a contiguous halves-add per block.
    order = np.argsort(rec, kind="stable")
    rs = rec[order]
    c = np.bincount(rec, minlength=N)
    cpairs = (c + 1) // 2
    P = int(cpairs.sum())
    assert 2 * P <= ST, (P, ST)
    pair_start = np.zeros(N, np.int64)
    np.cumsum(cpairs[:-1], out=pair_start[1:])
    run_start = np.zeros(N, np.int64)
    np.cumsum(c[:-1], out=run_start[1:])
    ranks = np.arange(E) - run_start[rs]
    q = pair_start[rs] + ranks // 2
    blk = q // 512
    off = q % 512
    slot = blk * GB + (ranks % 2) * 512 + off
    slot_edge = np.full(ST, -1, np.int64)
    slot_edge[slot] = order
    pad_mask = slot_edge < 0

    idx = np.maximum(slot_edge, 0)
    h_slot = h1[idx]
    h_slot[pad_mask] = NPF8(0)
    hp_slot = hp1[idx]
    hp_slot[pad_mask] = NPF8(0)

    wcat = np.concatenate([W2, Wp2], axis=0).astype(NPBF)
    biasT = np.stack([b2, bp2], axis=1).astype(np.float32)  # [H,2]

    in_maps = []
    for cidx in range(NCORES):
        sl = slice(cidx * EPS, (cidx + 1) * EPS)
        hT = np.ascontiguousarray(h_slot[sl].T)
        hpT = np.ascontiguousarray(hp_slot[sl].T)
        hcat = np.empty((NG, H, 2 * GE), NPF8)
        hcat[:, :, 0:GE] = hT.reshape(H, NG, GE).transpose(1, 0, 2)
        hcat[:, :, GE:2 * GE] = hpT.reshape(H, NG, GE).transpose(1, 0, 2)
        in_maps.append({"hcat": hcat.reshape(NG * H, 2 * GE),
                        "wcat": wcat, "biasT": biasT})

    if "nc" not in _CACHED:
        _CACHED["nc"] = _build_nc()
    nc = _CACHED["nc"]

    trace = bool(_CACHED.get("trace") or os.environ.get("KERNEL_TRACE"))
    res = run_bass_kernel_spmd(
        nc, in_maps, list(range(NCORES)), trace=trace,
        trace_cores=[0] if trace else None,
    )
    _CACHED["last_res"] = res

    OW = GE + GE // 2
    sp = np.empty((ST, H), np.float32)     # silu-path pre-acts, slot order
    tp = np.empty((ST // 2, H), np.float32)  # tanh pair sums, pair order
    for cidx in range(NCORES):
        oc = res.results[cidx]["ocat"].reshape(NG, H, OW)
        spT = oc[:, :, 0:GE].transpose(1, 0, 2).reshape(H, EPS)
        tpT = oc[:, :, GE:OW].transpose(1, 0, 2).reshape(H, EPS // 2)
        sp[cidx * EPS:(cidx + 1) * EPS] = spT.T.astype(np.float32)
        tp[cidx * EPS // 2:(cidx + 1) * EPS // 2] = tpT.T.astype(np.float32)

    # silu path: act on host, zero pads, fold slot pairs, segment-sum
    msg = _silu(sp)
    msg[pad_mask] = 0.0
    mv = msg.reshape(ST // GB, GB, H)
    msum = (mv[:, 0:512, :] + mv[:, 512:GB, :]).reshape(ST // 2, H)

    # tanh path: zero trailing dummy pairs, correct in-run pads
    tp[P:] = 0.0

    nz = c > 0
    seg = pair_start[nz]
    nnz = int(nz.sum())
    aggr = np.zeros((N, H), np.float32)
    aggr[nz] = np.add.reduceat(msum, seg, axis=0)[np.arange(nnz)]
    aggr_pe = np.zeros((N, H), np.float32)
    aggr_pe[nz] = np.add.reduceat(tp, seg, axis=0)[np.arange(nnz)]
    # each odd-count receiver has one empty slot whose tanh-path contribution
    # is tanh(0 @ Wp2 + bp2); remove it (silu-path pads were zeroed above)
    odd = (c & 1).astype(np.float32)
    aggr_pe -= odd[:, None] * np.tanh(bp2.astype(np.float32))[None, :]

    return x + aggr, pe_a + aggr_pe


# revision 8
# speedup vs baseline: 4.8544x; 1.0991x over previous
"""MPNN-LSPE layer on 8 trn2 NeuronCores.

Strategy (edge-parallel, per sharding hint): edges are sharded across the 8
cores.  The first MLP layer is linear, so it is algebraically restructured
into per-node projections computed once per node instead of once per edge:

    state @ W1 = A[send] + B[rec] + dist * w1e,   A = x@W1a + pe@W1b, ...

The host computes the node projections, gathers them per edge, applies the
first activation, and ships h1 = silu(p1), hp1 = tanh(pp1) feature-major in
fp8 (e4m3).  Edges are sorted by receiver and packed into even-sized
per-receiver slot runs so the device can pre-reduce pairs of messages that
share a receiver, cutting the output stream.  Each core then runs the
second half of both edge MLPs:

    silu path: matmul(W2, bf16) -> DVE +b2 -> pre-act out (host applies Silu)
    pe path:   matmul(Wp2)      -> ACT Tanh(psum + bp2) -> GPSIMD pair-add

Output per 2048-slot group: 2048 silu pre-act columns + 1024 tanh pair sums
(bf16), written on the ACT HWDGE ring while inputs stream on the SP ring.
The final segment-sum + residual runs on host (fp32).
"""

import os
import numpy as np
import ml_dtypes

import concourse.bass as bass
import concourse.mybir as mybir
import concourse.tile as tile
import bass_rust
from concourse.vector_clock import ScopedClock
from concourse.bass_utils import run_bass_kernel_spmd

N = 50000
E = 400000
H = 128
NCORES = 8
GE = 2048                 # slots per outer group
GB = 1024                 # slots per act block (= psum tile)
EPS = 26 * GE             # 53248 slots per core (fits E + even-padding)
NG = EPS // GE            # 26 groups
ST = NCORES * EPS         # total slots

F32 = mybir.dt.float32
BF16 = mybir.dt.bfloat16
FP8 = mybir.dt.float8e4

NPF8 = ml_dtypes.float8_e4m3
NPBF = ml_dtypes.bfloat16


def _patch_tail_drain():
    """Walrus rejects >2 sync waits on one instruction; the Tile tail drain
    accumulates one wait per outstanding sem. Spread them over SP nops."""
    def _split_drain_and_barrier(self, tick_clock, wait_clock):
        nc = self.nc
        spills = [nc.sync.nop(nofuse=True) for _ in range(24)]
        drain_inst = nc.sync.drain()
        wait_clock.add_sem_waits(
            drain_inst.ins, ScopedClock({None: tick_clock.global_clock})
        )
        si = drain_inst.ins.sync_info
        waits = list(si.on_wait) if si is not None else []
        if len(waits) > 1:
            si.on_wait = waits[:1]
            rest = waits[1:]
            assert len(rest) <= len(spills)
            for w, sp in zip(rest, spills):
                sp.ins.sync_info = bass_rust.SyncInfo(on_wait=[w], on_update=[])
        nc.all_engine_barrier()
        popped = nc._tile_sem_poison_stack.pop()
        assert popped is self._sem_poison
        nc.clear_and_free_semaphores(list(self.sems.allocated().values()))
        nc.all_engine_barrier()

    tile.TileContext._drain_and_barrier = _split_drain_and_barrier


def _split_excess_waits(nc, max_waits=1):
    """Walrus codegen caps embedded sync-wait commands per instruction; hoist
    excess waits onto same-engine no-ops inserted just before the inst."""
    for fn in nc.m.functions:
        for blk in fn.blocks:
            new_insts = []
            for inst in blk.instructions:
                si = inst.sync_info
                waits = list(si.on_wait) if si is not None else []
                if len(waits) > max_waits:
                    keep = waits[:max_waits]
                    rest = waits[max_waits:]
                    for k in range(0, len(rest), max_waits):
                        nop = mybir.InstNoOp(
                            name=nc.get_next_instruction_name(),
                            engine=inst.engine,
                            ins=[], outs=[],
                            sync_info=bass_rust.SyncInfo(
                                on_wait=rest[k:k + max_waits], on_update=[]
                            ),
                        )
                        new_insts.append(nop)
                    si.on_wait = keep
                new_insts.append(inst)
            blk.instructions = new_insts


def _build_nc():
    nc = bass.Bass()
    # hcat row-block g: [128 features, 2*GE] = h slots (0:GE) | hp slots
    hcat = nc.dram_tensor("hcat", [NG * H, 2 * GE], FP8, kind="ExternalInput")
    wcat = nc.dram_tensor("wcat", [2 * H, H], BF16, kind="ExternalInput")
    biasT = nc.dram_tensor("biasT", [H, 2], F32, kind="ExternalInput")
    # ocat row-block g: silu pre-acts (0:GE) | tanh pair sums (GE:GE+GE//2)
    OW = GE + GE // 2
    ocat = nc.dram_tensor("ocat", [NG * H, OW], BF16, kind="ExternalOutput")

    AF = mybir.ActivationFunctionType
    ADD = mybir.AluOpType.add

    with tile.TileContext(nc) as tc:
        with tc.tile_pool(name="consts", bufs=1) as cpool, \
             tc.tile_pool(name="io", bufs=6) as iopool, \
             tc.tile_pool(name="out", bufs=6) as outpool, \
             tc.tile_pool(name="mid", bufs=4) as midpool, \
             tc.tile_pool(name="psm", bufs=2, space="PSUM") as psm, \
             tc.tile_pool(name="psp", bufs=2, space="PSUM") as psp:

            w2 = cpool.tile([H, H], BF16, tag="w2")
            nc.sync.dma_start(out=w2[:], in_=wcat[0:H, :])
            wp2 = cpool.tile([H, H], BF16, tag="wp2")
            nc.sync.dma_start(out=wp2[:], in_=wcat[H:2 * H, :])
            bias = cpool.tile([H, 2], F32, tag="bias")
            nc.sync.dma_start(out=bias[:], in_=biasT[:, :])

            for g in range(NG):
                hin = iopool.tile([H, 2 * GE], FP8, tag="hin")
                nc.sync.dma_start(
                    out=hin[:], in_=hcat[g * H:(g + 1) * H, :]
                )
                oout = outpool.tile([H, OW], BF16, tag="oout")
                to = midpool.tile([H, GE], BF16, tag="to")

                for half in range(2):
                    mo = half * GB
                    # silu path: pre-act out (host applies Silu); +b2 on DVE
                    pm = psm.tile([H, GB], F32, tag="pm")
                    nc.tensor.matmul(
                        out=pm[:, 0:512], lhsT=w2[:],
                        rhs=hin[:, mo:mo + 512], start=True, stop=True)
                    nc.tensor.matmul(
                        out=pm[:, 512:GB], lhsT=w2[:],
                        rhs=hin[:, mo + 512:mo + GB], start=True, stop=True)
                    nc.vector.tensor_scalar_add(
                        oout[:, mo:mo + GB], pm[:], bias[:, 0:1])

                    # pe path: Tanh on ACT, then pair-add on GPSIMD
                    pp = psp.tile([H, GB], F32, tag="pp")
                    nc.tensor.matmul(
                        out=pp[:, 0:512], lhsT=wp2[:],
                        rhs=hin[:, GE + mo:GE + mo + 512],
                        start=True, stop=True)
                    nc.tensor.matmul(
                        out=pp[:, 512:GB], lhsT=wp2[:],
                        rhs=hin[:, GE + mo + 512:GE + mo + GB],
                        start=True, stop=True)
                    nc.scalar.activation(to[:, mo:mo + GB], pp[:],
                                         AF.Tanh, bias=bias[:, 1:2])
                    # pair q of this block = slots (q, q+512)
                    nc.gpsimd.tensor_tensor(
                        out=oout[:, GE + half * 512:GE + half * 512 + 512],
                        in0=to[:, mo:mo + 512],
                        in1=to[:, mo + 512:mo + GB],
                        op=ADD)

                # output DMA on gpsimd (SWDGE): separate queue rows from the
                # SP-ring input stream, and it follows this group's pair-adds
                # in the same engine FIFO, so the scalar engine never blocks
                # on a trigger that waits for DVE/GPSIMD completions.
                nc.gpsimd.dma_start(
                    out=ocat[g * H:(g + 1) * H, :], in_=oout[:]
                )

    _split_excess_waits(nc)
    return nc


_CACHED = {}


def _silu(v):
    return v / (1.0 + np.exp(-v))


def kernel(x, pos, pe, edge_index, W1, b1, W2, b2, Wp1, bp1, Wp2, bp2):
    _patch_tail_drain()

    x = np.asarray(x, np.float32)
    pos = np.asarray(pos, np.float32)
    pe_a = np.asarray(pe, np.float32)
    ei = np.asarray(edge_index)
    send = ei[0].astype(np.int64)
    rec = ei[1].astype(np.int64)
    W1 = np.asarray(W1, np.float32); b1 = np.asarray(b1, np.float32)
    W2 = np.asarray(W2, np.float32); b2 = np.asarray(b2, np.float32)
    Wp1 = np.asarray(Wp1, np.float32); bp1 = np.asarray(bp1, np.float32)
    Wp2 = np.asarray(Wp2, np.float32); bp2 = np.asarray(bp2, np.float32)

    dist = np.sqrt(((pos[send] - pos[rec]) ** 2).sum(axis=1)).astype(np.float32)

    # first (linear) MLP layers as per-node projections
    A = x @ W1[0:H] + pe_a @ W1[H:2 * H]
    B = x @ W1[2 * H:3 * H] + pe_a @ W1[3 * H:4 * H]
    Ap = pe_a @ Wp1[0:H]
    Bp = pe_a @ Wp1[H:2 * H]

    p1 = A[send] + B[rec]
    p1 += dist[:, None] * W1[4 * H][None, :]
    p1 += b1
    h1 = _silu(p1).astype(NPF8)
    del p1
    pp1 = Ap[send] + Bp[rec]
    pp1 += dist[:, None] * Wp1[2 * H][None, :]
    pp1 += bp1
    hp1 = np.tanh(pp1).astype(NPF8)
    del pp1

    # ---- receiver-sorted slot/pair layout ----
    # Each receiver's edges occupy ceil(c/2) consecutive PAIRS; pair q maps
    # to slots (o, o+512) of 1024-slot act block b, with q = b*512 + o, so
    # the device pair-add is a contiguous halves-add per block.
    order = np.argsort(rec, kind="stable")
    rs = rec[order]
    c = np.bincount(rec, minlength=N)
    cpairs = (c + 1) // 2
    P = int(cpairs.sum())
    assert 2 * P <= ST, (P, ST)
    pair_start = np.zeros(N, np.int64)
    np.cumsum(cpairs[:-1], out=pair_start[1:])
    run_start = np.zeros(N, np.int64)
    np.cumsum(c[:-1], out=run_start[1:])
    ranks = np.arange(E) - run_start[rs]
    q = pair_start[rs] + ranks // 2
    blk = q // 512
    off = q % 512
    slot = blk * GB + (ranks % 2) * 512 + off
    slot_edge = np.full(ST, -1, np.int64)
    slot_edge[slot] = order
    pad_mask = slot_edge < 0

    idx = np.maximum(slot_edge, 0)
    h_slot = h1[idx]
    h_slot[pad_mask] = NPF8(0)
    hp_slot = hp1[idx]
    hp_slot[pad_mask] = NPF8(0)

    wcat = np.concatenate([W2, Wp2], axis=0).astype(NPBF)
    biasT = np.stack([b2, bp2], axis=1).astype(np.float32)  # [H,2]

    in_maps = []
    for cidx in range(NCORES):
        sl = slice(cidx * EPS, (cidx + 1) * EPS)
        hT = np.ascontiguousarray(h_slot[sl].T)
        hpT = np.ascontiguousarray(hp_slot[sl].T)
        hcat = np.empty((NG, H, 2 * GE), NPF8)
        hcat[:, :, 0:GE] = hT.reshape(H, NG, GE).transpose(1, 0, 2)
        hcat[:, :, GE:2 * GE] = hpT.reshape(H, NG, GE).transpose(1, 0, 2)
        in_maps.append({"hcat": hcat.reshape(NG * H, 2 * GE),
                        "wcat": wcat, "biasT": biasT})

    if "nc" not in _CACHED:
        _CACHED["nc"] = _build_nc()
    nc = _CACHED["nc"]

    trace = bool(_CACHED.get("trace") or os.environ.get("KERNEL_TRACE"))
    res = run_bass_kernel_spmd(
        nc, in_maps, list(range(NCORES)), trace=trace,
        trace_cores=[0] if trace else None,
    )
    _CACHED["last_res"] = res

    OW = GE + GE // 2
    sp = np.empty((ST, H), np.float32)     # silu-path pre-acts, slot order
    tp = np.empty((ST // 2, H), np.float32)  # tanh pair sums, pair order
    for cidx in range(NCORES):
        oc = res.results[cidx]["ocat"].reshape(NG, H, OW)
        spT = oc[:, :, 0:GE].transpose(1, 0, 2).reshape(H, EPS)
        tpT = oc[:, :, GE:OW].transpose(1, 0, 2).reshape(H, EPS // 2)
        sp[cidx * EPS:(cidx + 1) * EPS] = spT.T.astype(np.float32)
        tp[cidx * EPS // 2:(cidx + 1) * EPS // 2] = tpT.T.astype(np.float32)

    # silu path: act on host, zero pads, fold slot pairs, segment-sum
    msg = _silu(sp)
    msg[pad_mask] = 0.0
    mv = msg.reshape(ST // GB, GB, H)
    msum = (mv[:, 0:512, :] + mv[:, 512:GB, :]).reshape(ST // 2, H)

    # tanh path: zero trailing dummy pairs, correct in-run pads
    tp[P:] = 0.0

    nz = c > 0
    seg = pair_start[nz]
    nnz = int(nz.sum())
    aggr = np.zeros((N, H), np.float32)
    aggr[nz] = np.add.reduceat(msum, seg, axis=0)[np.arange(nnz)]
    aggr_pe = np.zeros((N, H), np.float32)
    aggr_pe[nz] = np.add.reduceat(tp, seg, axis=0)[np.arange(nnz)]
    # each odd-count receiver has one empty slot whose tanh-path contribution
    # is tanh(0 @ Wp2 + bp2); remove it (silu-path pads were zeroed above)
    odd = (c & 1).astype(np.float32)
    aggr_pe -= odd[:, None] * np.tanh(bp2.astype(np.float32))[None, :]

    return x + aggr, pe_a + aggr_pe
